# revision 68
# baseline (speedup 1.0000x reference)
"""DGCN forward kernel for Trainium2, 8-core data-parallel over batch.

Reference computation (per batch b):
    x1_s = a_s[b]^T @ X          (X = x[b] viewed [N=512, T*C=768])
    x2_s = a_s[b]^T @ x1_s
    h = concat([X, x1_0, x2_0, x1_1, x2_1, x1_2, x2_2], channel)   # [N,T,448]
    y = h @ W^T + bias

We commute the (linear) 1x1-conv projection past the diffusions:
    Z_k = X @ W_k^T   (W_k = W[:, 64k:64(k+1)]), k = 0..6
    y[b] = Z0 + A0^T (Z1 + A0^T Z2) + A1^T (Z3 + A1^T Z4) + A2^T (Z5 + A2^T Z6) + b

fp8 acceleration (mode-gated): TRN2's PE runs float8e4 matmuls in DoubleRow
perf mode at 0.5 cycles/output-row with 2 k-subtiles per instruction (4x the
f32r FLOP rate). Quantization error analysis: A is uniform[0,1), so signals
amplify coherently (~N*mean) through a diffusion while independent
quantization errors amplify incoherently (~sqrt(N)).  The only dangerous
path is the coherent (DC) component of the *streamed* operand's error; we
remove it by decomposing A = 0.5*J + Atilde (J=ones) on the first diffusion
and adding the exact rank-1 term 0.5*colA_s (x) colsum(Z_even_s) -- computed
on the host in f32 from the same quantized operands the device uses -- via
extra K-rows of the bias matmul.  Tolerance 2e-2.

    U_s  = Atilde8_s^T @ Z8_even_s + Z_odd_s        (fp8 DR + f32 add)
    y[b] = Z0 + sum_s [ A8_s^T @ U8_s + 0.5 colA_s (x) c1_s ] + bias

Default mode "fp8zb" (measured ~114 us/pipeline, rel err 2.5e-3):
  - projection in bf16 (1 cyc/row like f32r, but FWL fast weight load and
    half the xt DMA bytes; fp8 projection is 2x faster still but its V/x
    quantization leaks ~3e-2 coherent error through the odd chunks -> only
    the diffusions run fp8 DoubleRow)
  - all 7 projection chunks land in ONE per-batch fp8 z tile
    [nt, q(7), t, o] (q order: evens, odds, z0) so each front unit drains
    PSUM with a single 896-elem copy, placed on ACT or DVE by weighted
    round-robin (act_share) -- DVE/ACT/PE end up balanced at ~75 us each
  - odd/Z0 adds stay on DVE as tensor_tensor (PSUM + fp8 SBUF operand);
    running them as PE identity matmuls ("fp8id") loses on HW to the extra
    DoubleRow LDWEIGHTS streams
  - input DMAs are prefetched two batches ahead (xt bufs=3, the 4.4us ab8
    DMA otherwise stalls ACT at every batch boundary)

Shapes (hardcoded): B=32, N=512, T=12, C=64, c_out=64, 8 cores x 4 batches.
"""

import numpy as np
import ml_dtypes

import concourse.bass as bass  # noqa: F401
import concourse.mybir as mybir
import concourse.tile as tile
from concourse import bacc
from concourse.bass_utils import run_bass_kernel_spmd

F32 = mybir.dt.float32
F32R = mybir.dt.float32r
F8 = mybir.dt.float8e4
BF16 = mybir.dt.bfloat16
NP_F8 = ml_dtypes.float8_e4m3
NP_BF16 = ml_dtypes.bfloat16
DR = mybir.MatmulPerfMode.DoubleRow

B, N, T, C = 32, 512, 12, 64
NCORES = 8
BPC = B // NCORES          # batches per core
NT = N // 128              # node tiles
TC = T * C                 # 768
HALF = TC // 2             # 384  (PSUM bank-sized slice of the (t,o) free dim)
NK = 7                     # projection channel blocks
# V block order in the projection output: evens (diffusion rhs), odds, k0
KORDER = [2, 4, 6, 1, 3, 5, 0]

# mode: "f32r" baseline | "fp8y" 2nd diffusion fp8 | "fp8uy" both diffusions
#       | "fp8all" + projection fp8 (DR, slow LDW) | "fp8z" unified z tile +
#       fused drains | "fp8zb" fp8z + bf16 projection (FWL, half xt DMA) |
#       "fp8zx" fp8z + fp8 projection (FAILS 2e-2 accuracy - do not use)
DEFAULT_MODE = "fp8zb"
DEFAULT_HOST_ROUND = True

# tunables (read at build time)
CFG = dict(
    ch=7,           # Y-chunk size (MMs per interleave unit)
    a_bufs=16,
    zse_bufs=2,
    zsl_bufs=8,
    z_bufs=2,
    act_share=0.84,  # fraction of flexible drains routed to ACT (rest DVE)
    u_bufs=7,
    y_bufs=5,
    rotate=0,       # software-pipeline fronts/loads across For_i trips
    front_lead=False,  # interleave: front units lead instead of Y units
    zsl_dve_frac=0,    # every Nth zsL copy goes to DVE instead of ACT (0=off)
    zse_act_frac=2,    # every Nth zsE copy goes to ACT instead of DVE (0=off)
    dma_probe=0,       # TIMING PROBE ONLY: halve big input DMAs (breaks math)
)


def build_program(rep=1, mode=DEFAULT_MODE, loop_iters=None):
    """Build + compile the per-core Bass program. rep>1 repeats the whole
    4-batch pipeline (python-unrolled); loop_iters wraps the body in an
    on-device For_i loop (for timing amortization)."""
    fp8_y = mode in ("fp8y", "fp8uy", "fp8all", "fp8id", "fp8z", "fp8zx", "fp8zb")
    fp8_u = mode in ("fp8uy", "fp8all", "fp8id", "fp8z", "fp8zx", "fp8zb")
    fp8_p = mode == "fp8all"
    # fp8id/fp8z/fp8zx: unified fp8 z tile, one fused 896-elem drain per front
    # unit, engine-flexible (DVE/ACT weighted round-robin). fp8id additionally
    # runs the odd/Z0 adds on the PE as fp8 identity matmuls (extra
    # LDWEIGHTS); fp8z keeps them as DVE tensor_tensor ops. fp8zx = fp8z with
    # the projection operands in fp8 WITHOUT DoubleRow (same 1 cyc/row on the
    # PE but 4x smaller xt DMA and FWL-fast weight loads on HW).
    idadd = mode in ("fp8id", "fp8z", "fp8zx", "fp8zb")
    idmm = mode == "fp8id"
    fp8_xp = mode == "fp8zx"
    bf16_xp = mode == "fp8zb"
    mm_dt = F32R
    xp_dt = F8 if fp8_xp else (BF16 if bf16_xp else mm_dt)

    def asf32(ap):
        return ap.bitcast(F32)

    nc = bacc.Bacc("TRN2", target_bir_lowering=False, debug=False)

    if fp8_p:
        # [32, ksub(2), h(2), j(6), n(512)] fp8
        xt_d = nc.dram_tensor("xt8", [BPC, 32, 2 * 2 * 6 * N], F8,
                              kind="ExternalInput")
        v_d = nc.dram_tensor("v8", [32, 2 * NK * 64], F8, kind="ExternalInput")
    else:
        xt_d = nc.dram_tensor("xt", [BPC, 128, 6 * N], xp_dt,
                              kind="ExternalInput")
        v_d = nc.dram_tensor("v2", [128, NK * 64], xp_dt, kind="ExternalInput")
    if not fp8_u:
        a_d = [
            nc.dram_tensor(f"a{s}", [BPC, NT, 128, N], mm_dt,
                           kind="ExternalInput")
            for s in range(3)
        ]
    if fp8_y and fp8_u:
        # one tensor: [a8_0|a8_1|a8_2|at8_0|at8_1|at8_2] -> single DMA/batch
        ab8_d = nc.dram_tensor("ab8", [BPC, 128, 6 * NT * N], F8,
                               kind="ExternalInput")
    elif fp8_y:
        a8_d = [
            nc.dram_tensor(f"a8_{s}", [BPC, 128, NT * N], F8,
                           kind="ExternalInput")
            for s in range(3)
        ]
    if fp8_u:
        # K=4 combined bias + rank-1 correction operands: [onescol|biasc1]
        ocbc_d = nc.dram_tensor("ocbc", [BPC, 4, N + TC], mm_dt,
                                kind="ExternalInput")
    else:
        ones_d = nc.dram_tensor("ones1", [1, 128], mm_dt, kind="ExternalInput")
        biasrow_d = nc.dram_tensor("biasrow", [1, TC], mm_dt,
                                   kind="ExternalInput")
    if idmm:
        # Two DoubleRow identity lhsTs: idA=[I;0] (odd adds, dummy ko=1 reads
        # the next written q chunk), idB=[0;I] (Z0 add, dummy ko=0 reads the
        # previous q chunk). Dummy reads must hit WRITTEN data: fp8 garbage
        # can decode as NaN and NaN*0 = NaN in the MAC.
        id8_d = nc.dram_tensor("id8", [128, 2 * 2 * 128], F8,
                               kind="ExternalInput")
    y_d = nc.dram_tensor("y", [BPC, N, TC], F32, kind="ExternalOutput")
    # z tile: [nt(NT), q(7), t(T), o(64)] fp8
    ZSZ = NT * 7 * TC

    with tile.TileContext(nc) as tc:
        pools = [
            tc.tile_pool(name="consts", bufs=1),
            tc.tile_pool(name="xt", bufs=4 if CFG["rotate"] else 3),
            tc.tile_pool(name="zsl", bufs=CFG["zsl_bufs"]),
            tc.tile_pool(name="u", bufs=6 if CFG["rotate"] else CFG["u_bufs"]),
            tc.tile_pool(name="y", bufs=4 if CFG["rotate"] else CFG["y_bufs"]),
            tc.tile_pool(name="psz", bufs=2, space="PSUM"),
            tc.tile_pool(name="psa", bufs=2, space="PSUM"),
        ]
        with (
            pools[0] as cpool,
            pools[1] as xt_pool,
            pools[2] as zsl_pool,
            pools[3] as u_pool,
            pools[4] as y_pool,
            pools[5] as psz_pool,
            pools[6] as psa_pool,
            tc.tile_pool(name="a", bufs=CFG["a_bufs"]) as a_pool,
            tc.tile_pool(name="a8", bufs=4 if CFG["rotate"] else 3) as a8_pool,
            tc.tile_pool(name="at8", bufs=3) as at8_pool,
            tc.tile_pool(name="zse", bufs=CFG["zse_bufs"]) as zse_pool,
            tc.tile_pool(name="z", bufs=CFG["z_bufs"]) as z_pool,
            tc.tile_pool(name="bc", bufs=4) as bc_pool,
        ):
            if fp8_p:
                v_sb = cpool.tile([32, 2 * NK * 64], F8, tag="v8")
            else:
                v_sb = cpool.tile([128, NK * 64], xp_dt, tag="v2")
            nc.sync.dma_start(v_sb[:], v_d.ap()[:])
            if idmm:
                id_sb = cpool.tile([128, 2 * 2 * 128], F8, tag="id8")
                nc.sync.dma_start(id_sb[:], id8_d.ap()[:])
                idA = id_sb[:, :256].rearrange("p (k c) -> p k c", k=2)
                idB = id_sb[:, 256:].rearrange("p (k c) -> p k c", k=2)
            if idadd:
                # weighted ACT/DVE round-robin for PSUM->SBUF drains
                drain_acc = [0.0]

                def drain_copy(out, in_):
                    drain_acc[0] += CFG["act_share"]
                    if drain_acc[0] >= 1.0:
                        drain_acc[0] -= 1.0
                        nc.scalar.copy(out, in_)
                    else:
                        nc.vector.tensor_copy(out=out, in_=in_)
            if not fp8_u:
                ones_sb = cpool.tile([1, 128], mm_dt, tag="ones")
                nc.sync.dma_start(ones_sb[:], ones_d.ap()[:])
                biasrow_sb = cpool.tile([1, TC], mm_dt, tag="biasrow")
                nc.sync.dma_start(biasrow_sb[:], biasrow_d.ap()[:])

            # per-batch-index live tile state (keyed by pipeline index)
            state = {}

            def emit_loads(i, b):
                if fp8_p:
                    xt_sb = xt_pool.tile([32, 2 * 2 * 6 * N], F8, tag="xt",
                                         name=f"xt_{i}")
                else:
                    xt_sb = xt_pool.tile([128, 6 * N], xp_dt, tag="xt",
                                         name=f"xt_{i}")
                if CFG["dma_probe"]:
                    nc.sync.dma_start(xt_sb[:, :xt_sb.shape[1] // 2],
                                      xt_d.ap()[b][:, :xt_sb.shape[1] // 2])
                else:
                    nc.sync.dma_start(xt_sb[:], xt_d.ap()[b])
                st = dict(xt=xt_sb, b=b)
                if not fp8_u:
                    a_sb = {}
                    for s in range(3):
                        for vt in range(NT):
                            at = a_pool.tile([128, N], mm_dt, tag="a",
                                             name=f"a_{i}_{s}_{vt}")
                            nc.sync.dma_start(at[:], a_d[s].ap()[b, vt])
                            a_sb[s, vt] = at
                    st["a"] = a_sb
                hp = (lambda ap: ap[:, :NT * N // 2]) if CFG["dma_probe"] \
                    else (lambda ap: ap)
                if fp8_y and fp8_u:
                    ab = a8_pool.tile([128, 6 * NT * N], F8, tag="a8",
                                      name=f"ab8_{i}")
                    nc.sync.dma_start(hp(ab[:]), hp(ab8_d.ap()[b]))
                    st["a8"] = [ab[:, s * NT * N:(s + 1) * NT * N]
                                for s in range(3)]
                    st["at8"] = [ab[:, (3 + s) * NT * N:(4 + s) * NT * N]
                                 for s in range(3)]
                elif fp8_y:
                    a8_sb = []
                    for s in range(3):
                        at = a8_pool.tile([128, NT * N], F8, tag="a8",
                                          name=f"a8_{i}_{s}")
                        nc.sync.dma_start(hp(at[:]), hp(a8_d[s].ap()[b]))
                        a8_sb.append(at[:])
                    st["a8"] = a8_sb
                if fp8_u:
                    ob = bc_pool.tile([4, N + TC], mm_dt, tag="oc",
                                      name=f"ocbc_{i}")
                    nc.sync.dma_start(ob[:], ocbc_d.ap()[b])
                    st["oc"] = ob[:, :N]
                    st["bc"] = ob[:, N:]
                state[i] = st

            def front_units_id(i):
                """fp8id projection units: 2 MMs + ONE fused 896-elem copy
                into the unified z tile [nt, q(7), t, o] (q order: evens,
                odds, z0 per KORDER)."""
                st = state[i]
                xt_sb = st["xt"]
                zt = z_pool.tile([128, ZSZ], F8, tag="z", name=f"z_{i}")
                st["z"] = zt
                z5 = zt[:, :NT * 7 * TC].rearrange(
                    "p (n q t o) -> p n q t o", n=NT, q=7, t=T)
                st["z5"] = z5
                # 2-free-dim views with contiguous inner slices (matmul rhs
                # APs must keep the streamed free dim flat)
                st["zq"] = zt[:, :NT * 7 * TC].rearrange(
                    "p (n f) -> p n f", n=NT)
                st["zq2"] = zt[:, :NT * 7 * TC].rearrange(
                    "p (n f) -> p n f", n=NT * 7)

                def unit(nt, j):
                    zp = psz_pool.tile([128, 1024], F32, tag="psz",
                                       name=f"zp_{i}_{nt}_{j}")
                    col = j * N + nt * 128
                    for h in range(2):
                        nc.tensor.matmul(
                            zp[:, h * 512:h * 512 + NK * 64],
                            lhsT=xt_sb[h * 64:(h + 1) * 64, col:col + 128],
                            rhs=v_sb[h * 64:(h + 1) * 64, :],
                            start=True, stop=True,
                        )
                    src = zp[:].rearrange("p (h f) -> p h f", h=2)[
                        :, :, :NK * 64].rearrange("p h (q o) -> p h q o", q=NK)
                    t0 = 2 * j
                    dst = z5[:, nt, :, t0:t0 + 2, :].rearrange(
                        "p q h o -> p h q o")
                    drain_copy(dst, src)

                return [(lambda nt=nt, j=j: unit(nt, j))
                        for nt in range(NT) for j in range(6)]

            def u_units_id(i):
                """fp8id first diffusion: U_s = At8_s^T Z_even + Z_odd with
                the Z_odd add done by an fp8 identity matmul into PSUM; the
                drain is a plain copy."""
                st = state[i]
                z5 = st["z5"]
                u8_sb = []
                st["u8"] = u8_sb
                for s in range(3):
                    ut = u_pool.tile([128, NT * TC], F8, tag="u",
                                     name=f"u_{i}_{s}")
                    u8_sb.append(ut)
                uW = [u8_sb[s][:].rearrange("p (w h f) -> p w h f",
                                            w=NT, h=2) for s in range(3)]

                def unit(s, wt):
                    up = psa_pool.tile([128, 1024], F32, tag="psa",
                                       name=f"up_{i}_{s}_{wt}")
                    at8 = st["at8"][s].rearrange("p (v w) -> p v w", v=NT)
                    zq, zq2 = st["zq"], st["zq2"]
                    for kp in range(2):
                        for h in range(2):
                            off = s * TC + h * HALF
                            nc.tensor.matmul(
                                up[:, h * 512:h * 512 + HALF],
                                lhsT=at8[:, 2 * kp:2 * kp + 2,
                                         wt * 128:(wt + 1) * 128],
                                rhs=zq[:, 2 * kp:2 * kp + 2,
                                       off:off + HALF],
                                start=(kp == 0),
                                stop=(not idmm and kp == 1),
                                perf_mode=DR,
                            )
                    if idmm:
                        for h in range(2):
                            zo = zq2[:, wt * 7 + 3 + s:wt * 7 + 5 + s,
                                     h * HALF:h * HALF + HALF]
                            nc.tensor.matmul(
                                up[:, h * 512:h * 512 + HALF],
                                lhsT=idA,
                                rhs=zo,
                                start=False, stop=(h == 1),
                                perf_mode=DR,
                            )
                        drain_copy(
                            uW[s][:, wt],
                            up[:].rearrange("p (h f) -> p h f",
                                            h=2)[:, :, :HALF],
                        )
                    else:
                        nc.vector.tensor_tensor(
                            uW[s][:, wt],
                            up[:].rearrange("p (h f) -> p h f",
                                            h=2)[:, :, :HALF],
                            zq2[:, wt * 7 + 3 + s, :].rearrange(
                                "p (h f) -> p h f", h=2),
                            mybir.AluOpType.add,
                        )

                return [(lambda s=s, wt=wt: unit(s, wt))
                        for s in range(3) for wt in range(NT)]

            def y_units_id(i):
                """fp8id second diffusion + combine: bias/rank-1 matmul, 12
                diffusion MMs, Z0 identity MM; plain-copy drain + DMA out."""
                st = state[i]
                b = st["b"]
                zt = st["z"]
                units = []
                for wt in range(NT):
                    tiles = {}

                    def alloc(wt=wt, tiles=tiles):
                        tiles["yt"] = y_pool.tile([128, TC], F32, tag="y",
                                                  name=f"y_{i}_{wt}")
                        tiles["yp"] = psa_pool.tile([128, 1024], F32,
                                                    tag="psa",
                                                    name=f"yp_{i}_{wt}")

                    mms = []
                    for h in range(2):
                        mms.append(("bias", h))
                    for s in range(3):
                        for kp in range(2):
                            for h in range(2):
                                mms.append((h, s, kp))
                    if idmm:
                        for h in range(2):
                            mms.append(("id", h))

                    def run_chunk(chunk, tiles=tiles, wt=wt):
                        yp = tiles["yp"]
                        for item in chunk:
                            if item[0] == "bias":
                                h = item[1]
                                nc.tensor.matmul(
                                    yp[:, h * 512:h * 512 + HALF],
                                    lhsT=st["oc"][:, wt * 128:(wt + 1) * 128],
                                    rhs=st["bc"][:, h * HALF:(h + 1) * HALF],
                                    start=True, stop=False,
                                )
                            elif item[0] == "id":
                                h = item[1]
                                z0 = st["zq2"][:, wt * 7 + 5:wt * 7 + 7,
                                               h * HALF:h * HALF + HALF]
                                nc.tensor.matmul(
                                    yp[:, h * 512:h * 512 + HALF],
                                    lhsT=idB,
                                    rhs=z0,
                                    start=False, stop=(h == 1),
                                    perf_mode=DR,
                                )
                            else:
                                h, s, kp = item
                                a8 = st["a8"][s].rearrange(
                                    "p (v w) -> p v w", v=NT)
                                u8 = st["u8"][s][:].rearrange(
                                    "p (v f) -> p v f", v=NT)
                                nc.tensor.matmul(
                                    yp[:, h * 512:h * 512 + HALF],
                                    lhsT=a8[:, 2 * kp:2 * kp + 2,
                                            wt * 128:(wt + 1) * 128],
                                    rhs=u8[:, 2 * kp:2 * kp + 2,
                                           h * HALF:h * HALF + HALF],
                                    start=False,
                                    stop=(not idmm and s == 2 and kp == 1),
                                    perf_mode=DR,
                                )

                    def finish(tiles=tiles, wt=wt, b=b):
                        yt, yp = tiles["yt"], tiles["yp"]
                        if idmm:
                            drain_copy(
                                yt[:].rearrange("p (h f) -> p h f", h=2),
                                yp[:].rearrange("p (h f) -> p h f",
                                                h=2)[:, :, :HALF],
                            )
                        else:
                            nc.vector.tensor_tensor(
                                yt[:].rearrange("p (h f) -> p h f", h=2),
                                yp[:].rearrange("p (h f) -> p h f",
                                                h=2)[:, :, :HALF],
                                st["zq2"][:, wt * 7 + 6, :].rearrange(
                                    "p (h f) -> p h f", h=2),
                                mybir.AluOpType.add,
                            )
                        nc.sync.dma_start(
                            y_d.ap()[b, wt * 128:(wt + 1) * 128, :], yt[:]
                        )

                    CH = CFG["ch"]
                    chunks = [mms[q:q + CH] for q in range(0, len(mms), CH)]
                    for ci, ch in enumerate(chunks):
                        first = (ci == 0)
                        last = (ci == len(chunks) - 1)
                        units.append(
                            (lambda ch=ch, first=first, last=last,
                                    alloc=alloc, run_chunk=run_chunk,
                                    finish=finish:
                             (alloc() if first else None,
                              run_chunk(ch),
                              finish() if last else None)))
                return units

            def front_units(i):
                """Projection work units (one t-pair each): 2 MMs + 2 copies.

                zsE free layout: (nt, ke, t, o); ke = 0,1,2 ~ k = 2,4,6
                zsL[nt] free layout: (kl, t, o); kl = 0..2 ~ k = 1,3,5; kl=3 ~ k=0
                """
                if idadd:
                    return front_units_id(i)
                st = state[i]
                xt_sb = st["xt"]
                zsl = []
                st["zsl"] = zsl
                zdt = F8 if fp8_u else mm_dt
                zdt_l = F8 if fp8_u else mm_dt
                zse8 = zse_pool.tile([128, NT * 3 * TC], zdt, tag="zse",
                                     name=f"zse_{i}")
                st["zse"] = zse8
                ze4 = zse8[:].rearrange("p (n k f) -> p n k f", n=NT, k=3)

                def unit(nt, j):
                    if j == 0:
                        zsl.append(zsl_pool.tile([128, 4 * TC], zdt_l,
                                                 tag="zsl",
                                                 name=f"zsl_{i}_{nt}"))
                    zl3 = zsl[nt][:].rearrange("p (k f) -> p k f", k=4)
                    zp = psz_pool.tile([128, 1024], F32, tag="psz",
                                       name=f"zp_{i}_{nt}_{j}")
                    if fp8_p:
                        x5 = xt_sb[:].rearrange(
                            "p (k h j n) -> p k h j n", k=2, h=2, j=6)
                        v3 = v_sb[:].rearrange("p (k f) -> p k f", k=2)
                        for h in range(2):
                            nc.tensor.matmul(
                                zp[:, h * 512:h * 512 + NK * 64],
                                lhsT=x5[:, :, h, j, nt * 128:nt * 128 + 128],
                                rhs=v3,
                                start=True, stop=True,
                                perf_mode=DR,
                            )
                    else:
                        col = j * N + nt * 128
                        for h in range(2):
                            nc.tensor.matmul(
                                zp[:, h * 512:h * 512 + NK * 64],
                                lhsT=xt_sb[h * 64:(h + 1) * 64, col:col + 128],
                                rhs=v_sb[h * 64:(h + 1) * 64, :],
                                start=True, stop=True,
                            )
                    zp4 = zp[:].rearrange("p (h f) -> p h f", h=2)
                    zpE = zp4[:, :, 0:192].rearrange("p h (k o) -> p h k o", k=3)
                    zpL = zp4[:, :, 192:448].rearrange("p h (k o) -> p h k o", k=4)
                    t0 = 2 * j
                    dstE = ze4[:, nt, :, t0 * 64:(t0 + 2) * 64].rearrange(
                        "p k (h o) -> p h k o", h=2)
                    dstL = zl3[:, :, t0 * 64:(t0 + 2) * 64].rearrange(
                        "p k (h o) -> p h k o", h=2)
                    fe = CFG["zse_act_frac"]
                    if fe and (nt * 6 + j) % fe == 0:
                        nc.scalar.copy(dstE, zpE)
                    else:
                        nc.vector.tensor_copy(out=dstE, in_=zpE)
                    fr = CFG["zsl_dve_frac"]
                    if fr and (nt * 6 + j) % fr == 0:
                        nc.vector.tensor_copy(out=dstL, in_=zpL)
                    else:
                        nc.scalar.copy(dstL, zpL)

                return [(lambda nt=nt, j=j: unit(nt, j))
                        for nt in range(NT) for j in range(6)]

            def u_units(i):
                """First diffusion: U_s = A_s^T Z_{2s+2} + Z_{2s+1}.

                Returns one unit per (s, wt) PSUM group so the caller can
                interleave projection work of the next batch into the PE
                gaps left while DVE drains each group's PSUM."""
                if idadd:
                    return u_units_id(i)
                st = state[i]
                zse8, zsl = st["zse"], st["zsl"]
                zeK = zse8[:].rearrange("p (n f) -> p n f", n=NT)
                u8_sb = []
                st["u8"] = u8_sb
                udt = F8 if fp8_y else mm_dt
                for s in range(3):
                    ut = u_pool.tile([128, NT * TC], udt, tag="u",
                                     name=f"u_{i}_{s}")
                    u8_sb.append(ut)
                uW = [u8_sb[s][:].rearrange("p (w h f) -> p w h f",
                                            w=NT, h=2) for s in range(3)]

                def unit(s, wt):
                    up = psa_pool.tile([128, 1024], F32, tag="psa",
                                       name=f"up_{i}_{s}_{wt}")
                    if fp8_u:
                        at8 = st["at8"][s].rearrange(
                            "p (v w) -> p v w", v=NT)
                        for h in range(2):
                            off_e = s * TC + h * HALF
                            for kp in range(2):
                                nc.tensor.matmul(
                                    up[:, h * 512:h * 512 + HALF],
                                    lhsT=at8[:, 2 * kp:2 * kp + 2,
                                             wt * 128:(wt + 1) * 128],
                                    rhs=zeK[:, 2 * kp:2 * kp + 2,
                                            off_e:off_e + HALF],
                                    start=(kp == 0), stop=(kp == 1),
                                    perf_mode=DR,
                                )
                    else:
                        a_sb = st["a"]
                        for h in range(2):
                            off_e = s * TC + h * HALF
                            for kt in range(NT):
                                nc.tensor.matmul(
                                    up[:, h * 512:h * 512 + HALF],
                                    lhsT=a_sb[s, kt][:, wt * 128:(wt + 1) * 128],
                                    rhs=ze4_slice(zse8, kt, off_e),
                                    start=(kt == 0), stop=(kt == NT - 1),
                                )
                    off_o = s * TC
                    zslop = zsl[wt][:, off_o:off_o + TC]
                    if not fp8_u:
                        zslop = asf32(zslop)
                    nc.vector.tensor_tensor(
                        uW[s][:, wt],
                        up[:].rearrange("p (h f) -> p h f", h=2)[:, :, :HALF],
                        zslop.rearrange("p (h f) -> p h f", h=2),
                        mybir.AluOpType.add,
                    )

                return [(lambda s=s, wt=wt: unit(s, wt))
                        for s in range(3) for wt in range(NT)]

            def ze4_slice(zse8, kt, off):
                z = zse8[:].rearrange("p (n f) -> p n f", n=NT)
                return z[:, kt, off:off + HALF]

            def y_units(i):
                """Second diffusion + combine, as interleavable chunks."""
                if idadd:
                    return y_units_id(i)
                st = state[i]
                zsl, b = st["zsl"], st["b"]
                units = []
                for wt in range(NT):
                    tiles = {}

                    def alloc(wt=wt, tiles=tiles):
                        tiles["yt"] = y_pool.tile([128, TC], F32, tag="y",
                                                  name=f"y_{i}_{wt}")
                        tiles["yp"] = psa_pool.tile([128, 1024], F32, tag="psa",
                                                    name=f"yp_{i}_{wt}")

                    # flat MM list for this wt, bias first per half
                    mms = []
                    if fp8_y:
                        for h in range(2):
                            mms.append(("bias", h))
                            for s in range(3):
                                for kp in range(2):
                                    mms.append((h, s, kp))
                    else:
                        for h in range(2):
                            mms.append(("bias", h))
                            for s in range(3):
                                for kt in range(NT):
                                    mms.append((h, s, kt))

                    def run_chunk(chunk, tiles=tiles, wt=wt):
                        yp = tiles["yp"]
                        for item in chunk:
                            if item[0] == "bias":
                                h = item[1]
                                if fp8_u:
                                    nc.tensor.matmul(
                                        yp[:, h * 512:h * 512 + HALF],
                                        lhsT=st["oc"][:, wt * 128:(wt + 1) * 128],
                                        rhs=st["bc"][:, h * HALF:(h + 1) * HALF],
                                        start=True, stop=False,
                                    )
                                else:
                                    nc.tensor.matmul(
                                        yp[:, h * 512:h * 512 + HALF],
                                        lhsT=ones_sb[:],
                                        rhs=biasrow_sb[:, h * HALF:(h + 1) * HALF],
                                        start=True, stop=False,
                                    )
                            elif fp8_y:
                                h, s, kp = item
                                a8 = st["a8"][s].rearrange(
                                    "p (v w) -> p v w", v=NT)
                                u8 = st["u8"][s][:].rearrange(
                                    "p (v f) -> p v f", v=NT)
                                nc.tensor.matmul(
                                    yp[:, h * 512:h * 512 + HALF],
                                    lhsT=a8[:, 2 * kp:2 * kp + 2,
                                            wt * 128:(wt + 1) * 128],
                                    rhs=u8[:, 2 * kp:2 * kp + 2,
                                           h * HALF:h * HALF + HALF],
                                    start=False,
                                    stop=(s == 2 and kp == 1),
                                    perf_mode=DR,
                                )
                            else:
                                h, s, kt = item
                                a_sb = st["a"]
                                u8 = st["u8"][s][:].rearrange(
                                    "p (v f) -> p v f", v=NT)
                                nc.tensor.matmul(
                                    yp[:, h * 512:h * 512 + HALF],
                                    lhsT=a_sb[s, kt][:, wt * 128:(wt + 1) * 128],
                                    rhs=u8[:, kt, h * HALF:(h + 1) * HALF],
                                    start=False,
                                    stop=(s == 2 and kt == NT - 1),
                                )

                    def finish(tiles=tiles, wt=wt, b=b):
                        yt, yp = tiles["yt"], tiles["yp"]
                        z0op = zsl[wt][:, 3 * TC:4 * TC]
                        if not fp8_u:
                            z0op = asf32(z0op)
                        nc.vector.tensor_tensor(
                            yt[:].rearrange("p (h f) -> p h f", h=2),
                            yp[:].rearrange("p (h f) -> p h f", h=2)[:, :, :HALF],
                            z0op.rearrange("p (h f) -> p h f", h=2),
                            mybir.AluOpType.add,
                        )
                        nc.sync.dma_start(
                            y_d.ap()[b, wt * 128:(wt + 1) * 128, :], yt[:]
                        )

                    CH = CFG["ch"]  # MMs per chunk
                    chunks = [mms[q:q + CH] for q in range(0, len(mms), CH)]
                    for ci, ch in enumerate(chunks):
                        first = (ci == 0)
                        last = (ci == len(chunks) - 1)
                        units.append(
                            (lambda ch=ch, first=first, last=last, alloc=alloc,
                                    run_chunk=run_chunk, finish=finish:
                             (alloc() if first else None,
                              run_chunk(ch),
                              finish() if last else None)))
                return units

            def interleave(ua, ub):
                """Proportionally merge two unit lists (ub slightly leading)."""
                out = []
                na, nb = len(ua), len(ub)
                ia = ib = 0
                while ia < na or ib < nb:
                    if ib < nb and (ia >= na or ib * na <= ia * nb):
                        out.append(ub[ib]); ib += 1
                    else:
                        out.append(ua[ia]); ia += 1
                return out

            n = rep * BPC

            def emit_prologue():
                emit_loads(0, 0)
                if n > 1:
                    emit_loads(1, 1 % BPC)
                fu = front_units(0)
                for f in fu:
                    f()

            def emit_body():
                rot = CFG["rotate"] and loop_iters is not None
                for i in range(n):
                    k = (i + 2) % n if rot else i + 2
                    if k < n and k not in state:
                        emit_loads(k, k % BPC)
                    uy = u_units(i) + y_units(i)
                    if rot:
                        fu = front_units((i + 1) % n)
                    else:
                        fu = front_units(i + 1) if i + 1 < n else []
                    merged = (interleave(uy, fu) if CFG["front_lead"]
                              else interleave(fu, uy))
                    for f in merged:
                        f()
                    state.pop(i)

            if loop_iters is None:
                emit_prologue()
                emit_body()
            elif CFG["rotate"]:
                # software-pipelined: prologue outside, wraparound body
                emit_prologue()
                with tc.For_i(0, loop_iters, 1,
                              hint_engines=(mybir.EngineType.PE,
                                            mybir.EngineType.DVE,
                                            mybir.EngineType.SP,
                                            mybir.EngineType.Activation,
                                            mybir.EngineType.Pool)):
                    emit_body()
            else:
                # full pipeline per trip (honest marginal-time measurement)
                with tc.For_i(0, loop_iters, 1,
                              hint_engines=(mybir.EngineType.PE,
                                            mybir.EngineType.DVE,
                                            mybir.EngineType.SP,
                                            mybir.EngineType.Activation,
                                            mybir.EngineType.Pool)):
                    emit_prologue()
                    emit_body()

    nc.compile()
    return nc


def tf32_round(arr):
    """Round fp32 to TF32 (10-bit mantissa), round-to-nearest-even."""
    u = np.ascontiguousarray(arr).view(np.uint32)
    lsb = (u >> np.uint32(13)) & np.uint32(1)
    r = u + np.uint32(0x0FFF) + lsb
    return (r & np.uint32(0xFFFFE000)).view(np.float32)


def q8(arr):
    return np.ascontiguousarray(np.asarray(arr, np.float32)).astype(NP_F8)


def qb(arr):
    return np.ascontiguousarray(np.asarray(arr, np.float32)).astype(NP_BF16)


def prep_inputs(x, a0, a1, a2, W, b, mode=DEFAULT_MODE,
                host_round=DEFAULT_HOST_ROUND):
    """Host-side shard + repack. Returns per-core in_maps."""
    fp8_y = mode in ("fp8y", "fp8uy", "fp8all", "fp8id", "fp8z", "fp8zx", "fp8zb")
    fp8_u = mode in ("fp8uy", "fp8all", "fp8id", "fp8z", "fp8zx", "fp8zb")
    fp8_p = mode == "fp8all"
    idmm = mode == "fp8id"
    fp8_xp = mode == "fp8zx"
    bf16_xp = mode == "fp8zb"
    x = np.ascontiguousarray(np.asarray(x, dtype=np.float32))
    aa = [np.ascontiguousarray(np.asarray(a, dtype=np.float32))
          for a in (a0, a1, a2)]
    W = np.asarray(W, dtype=np.float32)
    b = np.asarray(b, dtype=np.float32)

    rnd = tf32_round if host_round else (lambda v: v)

    # V[c, q*64+o] = W[o, KORDER[q]*64+c]
    Vk = W.reshape(64, NK, 64).transpose(2, 1, 0)        # [c, k, o]
    V = Vk[:, KORDER, :].reshape(64, NK * 64)
    if fp8_p:
        # v8[p, ksub, ko] = V[ksub*32+p, ko]
        v8 = q8(V.reshape(2, 32, NK * 64).transpose(1, 0, 2).reshape(
            32, 2 * NK * 64))
    elif fp8_xp:
        v2 = q8(np.ascontiguousarray(np.concatenate([V, V], axis=0)))
    elif bf16_xp:
        v2 = qb(np.ascontiguousarray(np.concatenate([V, V], axis=0)))
    else:
        v2 = rnd(np.ascontiguousarray(np.concatenate([V, V], axis=0)))
    ones1 = np.ones((1, 128), dtype=np.float32)
    biasrow = rnd(np.ascontiguousarray(np.tile(b, T)[None, :]))
    if idmm:
        id8 = np.zeros((128, 512), dtype=NP_F8)
        id8[np.arange(128), np.arange(128)] = 1.0          # idA: [I;0]
        id8[np.arange(128), 256 + 128 + np.arange(128)] = 1.0  # idB: [0;I]

    in_maps = []
    for ci in range(NCORES):
        sl = slice(ci * BPC, (ci + 1) * BPC)
        xs = x[sl]  # [BPC, N, T, C]
        m = {}
        if idmm:
            m["id8"] = id8
        if fp8_p:
            m["v8"] = v8
            # xt8[b, p, (ksub, h, j, n)] = x[b, n, 2j+h, ksub*32+p]
            xt8 = xs.reshape(BPC, N, 6, 2, 2, 32).transpose(0, 5, 4, 3, 2, 1)
            m["xt8"] = q8(np.ascontiguousarray(xt8).reshape(
                BPC, 32, 2 * 2 * 6 * N))
        else:
            # xt[b, h*64+c, j*512+n] = x[b, n, 2j+h, c]
            xt = np.ascontiguousarray(
                xs.reshape(BPC, N, 6, 2, C).transpose(0, 3, 4, 2, 1)
            ).reshape(BPC, 128, 6 * N)
            m["xt"] = (q8(xt) if fp8_xp else
                       qb(xt) if bf16_xp else rnd(xt))
            m["v2"] = v2
        if not fp8_u:
            m["ones1"] = ones1
            m["biasrow"] = biasrow
            for s in range(3):
                m[f"a{s}"] = rnd(np.ascontiguousarray(
                    aa[s][sl].reshape(BPC, NT, 128, N)))
        if fp8_y and fp8_u:
            ab = np.empty((BPC, 128, 6 * NT * N), NP_F8)
            for s in range(3):
                # a8[b, p, vt*N + w] = a[b, vt*128+p, w]
                a8 = aa[s][sl].reshape(BPC, NT, 128, N).transpose(0, 2, 1, 3)
                ab[:, :, s * NT * N:(s + 1) * NT * N] = q8(
                    np.ascontiguousarray(a8).reshape(BPC, 128, NT * N))
                at8 = (aa[s][sl] - np.float32(0.5)).reshape(
                    BPC, NT, 128, N).transpose(0, 2, 1, 3)
                ab[:, :, (3 + s) * NT * N:(4 + s) * NT * N] = q8(
                    np.ascontiguousarray(at8).reshape(BPC, 128, NT * N))
            m["ab8"] = ab
        elif fp8_y:
            for s in range(3):
                a8 = aa[s][sl].reshape(BPC, NT, 128, N).transpose(0, 2, 1, 3)
                m[f"a8_{s}"] = q8(np.ascontiguousarray(a8).reshape(
                    BPC, 128, NT * N))
        if fp8_u:
            # exact host-side rank-1 correction operands (f32). For fp8zx the
            # correction must match the DEVICE's Z_even = q8(X) @ q8(V): use
            # the quantized x and V blocks so 0.5*J^T Z_e cancels exactly.
            if fp8_xp:
                colsX = q8(x[sl]).astype(np.float32).sum(axis=1)  # [BPC,T,C]
            elif bf16_xp:
                colsX = qb(x[sl]).astype(np.float32).sum(axis=1)
            else:
                colsX = x[sl].sum(axis=1)                    # [BPC, T, C]
            ob = np.empty((BPC, 4, N + TC), np.float32)
            ob[:, 0, :N] = 1.0
            ob[:, 0, N:] = np.tile(b, T)[None]
            for s in range(3):
                ob[:, 1 + s, :N] = aa[s][sl].sum(axis=1)  # colA_s [BPC, N]
                k = 2 * s + 2
                if fp8_xp or bf16_xp:
                    # V[c, s*64+o] holds q-block s = channel block k=2s+2
                    qf = q8 if fp8_xp else qb
                    We = qf(V[:, 64 * s:64 * (s + 1)]).astype(np.float32)
                    c1 = np.einsum('btc,co->bto', colsX, We)
                else:
                    c1 = np.einsum('btc,oc->bto', colsX,
                                   W[:, 64 * k:64 * (k + 1)])
                ob[:, 1 + s, N:] = 0.5 * c1.reshape(BPC, TC)
            m["ocbc"] = rnd(np.ascontiguousarray(ob))
        in_maps.append(m)
    return in_maps


def gather_output(results):
    """results: list of per-core {'y': [BPC, N, TC]} -> [B, N, T, C]."""
    ys = [results[ci]["y"].reshape(BPC, N, T, C) for ci in range(NCORES)]
    return np.ascontiguousarray(np.concatenate(ys, axis=0))


_PROGRAM_CACHE = {}


def kernel(x, a0, a1, a2, W, b):
    key = (1, DEFAULT_MODE)
    if key not in _PROGRAM_CACHE:
        _PROGRAM_CACHE[key] = build_program(rep=key[0], mode=key[1])
    nc = _PROGRAM_CACHE[key]
    in_maps = prep_inputs(x, a0, a1, a2, W, b, mode=DEFAULT_MODE)
    res = run_bass_kernel_spmd(nc, in_maps, core_ids=list(range(NCORES)))
    return gather_output(res.results)


if __name__ == "__main__":
    rng = np.random.default_rng(0)
    x = rng.standard_normal((B, N, T, C), dtype=np.float32)
    a0 = rng.random((B, N, N), dtype=np.float32)
    a1 = rng.random((B, N, N), dtype=np.float32)
    a2 = rng.random((B, N, N), dtype=np.float32)
    W = (rng.standard_normal((64, 448), dtype=np.float32) * 0.05).astype(np.float32)
    b = (rng.standard_normal((64,), dtype=np.float32) * 0.05).astype(np.float32)
    y = kernel(x, a0, a1, a2, W, b)
    print("y shape", y.shape, "mean", y.mean())

